# revision 19
# baseline (speedup 1.0000x reference)
"""Single-head attention (B=8, S=2048, H=768, D=64) on 8 TRN2 NeuronCores.

Data-parallel over batch: core b computes batch element b end to end; no
collectives. Host pre-transposes Q/K/V to [H, S] bf16 so every matmul
contraction lands on the partition axis.

Per-core dataflow (matmuls bf16 x bf16 -> f32 PSUM):
  warmup MMs on the identity flip the PE HAM clock-gate to 2.4 GHz while
    the first input quarters stream in.
  [qT; kT][128, 512] = [Wq|Wk]^T @ [queryT, keyT]  per 512-col quarter via
    col-packed concurrent matmul pairs; bias, then PE identity-matmuls
    duplicate qT/kT into both partition halves (qqT/kkT) so scores can
    alternate PE row groups (hides LDWEIGHTS).
  S^T tile [sk=128, sq=1024] per (t, half) in PSUM; P^T = exp(S^T/8 + mask)
    straight to bf16 SBUF (ACT engine saturated ~35us = the floor).
  O^T accumulation: per (t, half) a col-packed concurrent pair
    (M=64 each, cols 0-63 / 64-127) multiplies V^T_t against the two
    512-col chunks of P^T; denominators via 4x col-packed ones[128,32]
    matmuls accumulated over t.
  Unnormalized O^T chunks + denominator rows DMA out as [128, 1536] f32;
  host does the divide + transpose (cheap numpy).
"""

import os
from contextlib import ExitStack

import numpy as np
import ml_dtypes

import concourse.bass as bass
import concourse.mybir as mybir
import concourse.tile as tile
from concourse import bacc
from concourse.bass_utils import run_bass_kernel_spmd

S, H, D = 2048, 768, 64
P = 128
NT = S // P      # 16 sk tiles
HT = H // P      # 6 h tiles
CH = 512         # sq chunk (quarter) = matmul free dim = PSUM bank
NCH = S // CH    # 4
BF = mybir.dt.bfloat16
F32 = mybir.dt.float32
AF = mybir.ActivationFunctionType

LAST_RESULT = None  # BassKernelResults of the most recent run (for test.py)


def _build(debug=False):
    nc = bacc.Bacc()
    qT_d = nc.declare_dram_parameter("qT", [H, S], BF, isOutput=False)
    kT_d = nc.declare_dram_parameter("kT", [H, S], BF, isOutput=False)
    vT_d = nc.declare_dram_parameter("vT", [H, S], BF, isOutput=False)
    # weights host-prepacked to [p, kind, h, n] so the DMA is contiguous
    wqk_d = nc.declare_dram_parameter("wqk", [P, HT * P], BF, isOutput=False)
    wvv_d = nc.declare_dram_parameter("wvv", [P, HT * P], BF, isOutput=False)
    id_d = nc.declare_dram_parameter("ident", [P, P], BF, isOutput=False)
    bqk_d = nc.declare_dram_parameter("bqk", [P, 1], F32, isOutput=False)
    bvv_d = nc.declare_dram_parameter("bvv", [P, 1], F32, isOutput=False)
    mb_d = nc.declare_dram_parameter("mb", [P, NT], F32, isOutput=False)
    o_d = nc.declare_dram_parameter("o", [P, 3 * CH], F32, isOutput=True)

    with ExitStack() as ctx:
        tc = ctx.enter_context(tile.TileContext(nc))
        consts = ctx.enter_context(tc.tile_pool(name="consts", bufs=1))
        stqk = ctx.enter_context(tc.tile_pool(name="stqk", bufs=4 * HT))
        stv = ctx.enter_context(tc.tile_pool(name="stv", bufs=2 * HT))
        persist = ctx.enter_context(tc.tile_pool(name="persist", bufs=1))
        ppool = ctx.enter_context(tc.tile_pool(name="ppool", bufs=2 * NT))
        psc = ctx.enter_context(tc.tile_pool(name="psc", bufs=1, space="PSUM"))
        psw = ctx.enter_context(tc.tile_pool(name="psw", bufs=1, space="PSUM"))
        psav = ctx.enter_context(tc.tile_pool(name="psav", bufs=1, space="PSUM"))

        # ---- weights + constants first so the PE warmup + first projection
        # can start as early as possible ----
        w_sb = consts.tile([P, 2, HT, P], BF, tag="w")  # wqk | wvv h-tiles
        ident_bf = consts.tile([P, P], BF, tag="ident_bf")
        nc.sync.dma_start(out=ident_bf, in_=id_d[:, :])
        nc.sync.dma_start(
            out=w_sb[:, 0, :, :],
            in_=wqk_d[:, :].rearrange("p (t n) -> p t n", t=HT),
        )
        nc.sync.dma_start(
            out=w_sb[:, 1, :, :],
            in_=wvv_d[:, :].rearrange("p (t n) -> p t n", t=HT),
        )
        bqk_sb = consts.tile([P, 1], F32, tag="bqk")
        nc.sync.dma_start(out=bqk_sb, in_=bqk_d[:, :])
        bvv_sb = consts.tile([P, 1], F32, tag="bvv")
        nc.sync.dma_start(out=bvv_sb, in_=bvv_d[:, :])
        mb_sb = consts.tile([P, NT], F32, tag="mb")
        nc.sync.dma_start(out=mb_sb, in_=mb_d[:, :])
        ones32 = consts.tile([P, 32], BF, tag="ones32")
        nc.vector.memset(ones32, 1.0)

        # ---- input staging: q/k in 512-col quarters (quarter == proj
        # chunk, so the first projection starts after ~1.5MB), v in halves
        # on the ACT HWDGE queue (consumed later) ----
        # halves = 2KB per-partition DMA lines (full HBM rate); projection
        # consumes 512-col quarter slices of them. k on the SP queue, q
        # then v on the ACT queue so k/q stream in parallel.
        st_q, st_k, st_v = {}, {}, {}
        HS = S // 2
        for hf in range(2):
            for h in range(HT):
                st = stqk.tile([P, HS], BF, tag="stqk", name=f"st_k{h}_{hf}")
                nc.sync.dma_start(
                    out=st, in_=kT_d[h * P : (h + 1) * P, hf * HS : (hf + 1) * HS]
                )
                for c in (2 * hf, 2 * hf + 1):
                    st_k[h, c] = st[:, (c % 2) * CH : (c % 2 + 1) * CH]
        for hf in range(2):
            for h in range(HT):
                st = stqk.tile([P, HS], BF, tag="stqk", name=f"st_q{h}_{hf}")
                nc.scalar.dma_start(
                    out=st, in_=qT_d[h * P : (h + 1) * P, hf * HS : (hf + 1) * HS]
                )
                for c in (2 * hf, 2 * hf + 1):
                    st_q[h, c] = st[:, (c % 2) * CH : (c % 2 + 1) * CH]
        for half in range(2):
            for h in range(HT):
                st = stv.tile([P, HS], BF, tag="stv", name=f"st_v{h}_{half}")
                nc.gpsimd.dma_start(
                    out=st,
                    in_=vT_d[h * P : (h + 1) * P, half * HS : (half + 1) * HS],
                )
                st_v[h, half] = st

        # ---- persistent SBUF tensors ----
        qqT_sb = persist.tile([P, S], BF, tag="qqT")  # qT in both halves
        kkT_sb = persist.tile([P, S], BF, tag="kkT")  # kT in both halves
        vT2_sb = persist.tile([P, S // 2], BF, tag="vT2")  # vT chunk pairs
        vE_sb = persist.tile([P, NT * D], BF, tag="vE")  # V tiles [sk, d]
        osb = persist.tile([P, 3 * CH], F32, tag="osb")  # output staging

        # ---- PE warmup: back-to-back dummy matmuls flip the HAM clock
        # gate to 2.4 GHz during the DMA lead-in (~3.4us of activity) ----
        warm = psw.tile([P, CH], F32, tag="aux", name="warm")
        for i in range(32):
            nc.tensor.matmul(
                warm[:, :P],
                lhsT=ident_bf,
                rhs=ident_bf,
                start=True,
                stop=True,
                skip_group_check=True,
            )

        # ---- helper blocks ----
        def qk_proj_chunk(c):
            """col-packed concurrent q/k projection for sq quarter c, then
            PE identity-matmul partition duplication for that quarter."""
            pp = psav.tile(
                [P, CH], F32, tag=f"av{(c % 2) * 2}{(c % 2) * 2 + 1}",
                name=f"pp{c}",
            )
            for h in range(HT):
                nc.tensor.matmul(
                    pp[:D, :],
                    lhsT=w_sb[:, 0, h, :D],
                    rhs=st_q[h, c],
                    start=(h == 0),
                    stop=(h == HT - 1),
                    tile_position=(0, 0),
                    skip_group_check=True,
                )
                nc.tensor.matmul(
                    pp[D:, :],
                    lhsT=w_sb[:, 0, h, D:],
                    rhs=st_k[h, c],
                    start=(h == 0),
                    stop=(h == HT - 1),
                    tile_position=(0, D),
                    skip_group_check=True,
                )
            ch = slice(c * CH, (c + 1) * CH)
            nc.vector.tensor_scalar_add(
                out=qqT_sb[:D, ch], in0=pp[:D, :], scalar1=bqk_sb[:D, :]
            )
            nc.vector.tensor_scalar_add(
                out=kkT_sb[D:, ch], in0=pp[D:, :], scalar1=bqk_sb[D:, :]
            )
            pd = psw.tile([P, CH], F32, tag="aux" if c % 2 else "den",
                          name=f"pd{c}")
            nc.tensor.matmul(
                pd[D:, :],
                lhsT=ident_bf[:D, :D],
                rhs=qqT_sb[:D, ch],
                start=True,
                stop=True,
                tile_position=(0, D),
                skip_group_check=True,
            )
            nc.tensor.matmul(
                pd[:D, :],
                lhsT=ident_bf[D:, D:],
                rhs=kkT_sb[D:, ch],
                start=True,
                stop=True,
                tile_position=(D, 0),
                skip_group_check=True,
            )
            nc.vector.tensor_copy(out=qqT_sb[D:, ch], in_=pd[D:, :])
            nc.vector.tensor_copy(out=kkT_sb[:D, ch], in_=pd[:D, :])

        pth = {}

        def scores_exp(t, half):
            """scores for sk-tile t over sq half (row group alternates with
            t to hide LDWEIGHTS), exp straight into a bf16 tile."""
            lo, hi = (0, D) if t % 2 == 0 else (D, P)
            ps = psc.tile([P, 2 * CH], F32, tag=f"sc{t % 2}",
                          name=f"ps{t}_{half}")
            for sub in range(2):
                c = 2 * half + sub
                nc.tensor.matmul(
                    ps[:, sub * CH : (sub + 1) * CH],
                    lhsT=kkT_sb[lo:hi, t * P : (t + 1) * P],
                    rhs=qqT_sb[lo:hi, c * CH : (c + 1) * CH],
                    start=True,
                    stop=True,
                    tile_position=(lo, 0),
                    skip_group_check=True,
                )
            pt = ppool.tile([P, 2 * CH], BF, tag="pT", name=f"pt{t}_{half}")
            nc.scalar.activation(
                out=pt,
                in_=ps,
                func=AF.Exp,
                bias=mb_sb[:, t : t + 1],
                scale=0.125,
            )
            pth[t, half] = pt

        pav = {}

        def vE_slice(t):
            """vE block layout: transpose j holds tiles (k, k+4) side by
            side; see v_proj."""
            b = (t // 8) * 4 + (t % 4)
            half = (t % 8) // 4
            return vE_sb[:, b * P + half * D : b * P + half * D + D]

        def av(t, half):
            """col-packed concurrent O^T accumulation pair for sk-tile t:
            chunk 2*half -> partitions 0:64, chunk 2*half+1 -> 64:128."""
            key = f"av{2 * half}{2 * half + 1}"
            if half not in pav:
                pav[half] = psav.tile([P, CH], F32, tag=key, name=key)
            vt = vE_slice(t)
            nc.tensor.matmul(
                pav[half][:D, :],
                lhsT=vt,
                rhs=pth[t, half][:, :CH],
                start=(t == 0),
                stop=(t == NT - 1),
                tile_position=(0, 0),
                skip_group_check=True,
            )
            nc.tensor.matmul(
                pav[half][D:, :],
                lhsT=vt,
                rhs=pth[t, half][:, CH:],
                start=(t == 0),
                stop=(t == NT - 1),
                tile_position=(0, D),
                skip_group_check=True,
            )

        pden = [None]

        def den4(t):
            """4x col-packed concurrent denominator matmuls: chunk ci's
            softmax denominator accumulates in partitions 32ci:32ci+32."""
            if pden[0] is None:
                pden[0] = psw.tile([P, CH], F32, tag="den", name="pden")
            for ci in range(NCH):
                nc.tensor.matmul(
                    pden[0][32 * ci : 32 * (ci + 1), :],
                    lhsT=ones32[:, :],
                    rhs=pth[t, ci // 2][:, (ci % 2) * CH : (ci % 2 + 1) * CH],
                    start=(t == 0),
                    stop=(t == NT - 1),
                    tile_position=(0, 32 * ci),
                    skip_group_check=True,
                )

        def v_proj(u):
            """v projection chunk-pair u (chunks 2u -> rows 0:64,
            2u+1 -> rows 64:128), then PE-transposes [128,128] blocks of
            vT2 into vE (each block = vE tiles k and k+4 side by side)."""
            pv = psw.tile([P, CH], F32, tag="den" if u == 0 else "aux",
                          name=f"pv{u}")
            for h in range(HT):
                nc.tensor.matmul(
                    pv[:D, :],
                    lhsT=w_sb[:, 1, h, :D],
                    rhs=st_v[h, u][:, :CH],
                    start=(h == 0),
                    stop=(h == HT - 1),
                    tile_position=(0, 0),
                    skip_group_check=True,
                )
                nc.tensor.matmul(
                    pv[D:, :],
                    lhsT=w_sb[:, 1, h, D:],
                    rhs=st_v[h, u][:, CH:],
                    start=(h == 0),
                    stop=(h == HT - 1),
                    tile_position=(0, D),
                    skip_group_check=True,
                )
            nc.vector.tensor_scalar_add(
                out=vT2_sb[:, u * CH : (u + 1) * CH], in0=pv, scalar1=bvv_sb
            )
            for j in range(4):
                tag = ("aux", "den")[(j + u) % 2]
                pt = psw.tile([P, P], BF, tag=tag, name=f"ptv{u}_{j}")
                nc.tensor.transpose(
                    pt,
                    in_=vT2_sb[:, u * CH + j * P : u * CH + (j + 1) * P],
                    identity=ident_bf,
                )
                b = 4 * u + j
                nc.vector.tensor_copy(
                    out=vE_sb[:, b * P : (b + 1) * P], in_=pt
                )

        # ---- schedule (program order == Tile priority) ----
        # phase A: project q/k quarters, stream scores+exp for sq half 0,
        # slot v-projection + first-half AV under the exp umbrella.
        qk_proj_chunk(0)
        qk_proj_chunk(1)
        for t in range(4):
            scores_exp(t, 0)
        qk_proj_chunk(2)
        qk_proj_chunk(3)
        for t in range(4, 8):
            scores_exp(t, 0)
        v_proj(0)
        for t in range(4):
            av(t, 0)
        for t in range(8, 12):
            scores_exp(t, 0)
        for t in range(4, 8):
            av(t, 0)
        v_proj(1)
        for t in range(12, NT):
            scores_exp(t, 0)
        for t in range(8, NT):
            av(t, 0)

        # phase B: sq half 1 + denominators. Pairs of t keep the PE in
        # coarse bursts (scores t,t+1 back-to-back, then av+den for both)
        # so the HAM clock gate stays at 2.4 GHz.
        for tp in range(0, NT, 2):
            scores_exp(tp, 1)
            scores_exp(tp + 1, 1)
            av(tp, 1)
            den4(tp)
            av(tp + 1, 1)
            den4(tp + 1)

        # ---- epilogue: stage unnormalized O^T + denominators, DMA out;
        # the host divides and transposes ----
        nc.vector.tensor_copy(out=osb[:, 0:CH], in_=pav[0])
        nc.sync.dma_start(out=o_d[:, 0:CH], in_=osb[:, 0:CH])
        nc.vector.tensor_copy(out=osb[:, CH : 2 * CH], in_=pav[1])
        nc.vector.tensor_copy(out=osb[:, 2 * CH : 3 * CH], in_=pden[0])
        nc.sync.dma_start(out=o_d[:, CH : 3 * CH], in_=osb[:, CH : 3 * CH])

    return nc


_NC = None


def kernel(query, key, value, mask, Wq, bq, Wk, bk, Wv, bv):
    global _NC, LAST_RESULT
    bf16 = ml_dtypes.bfloat16
    B = query.shape[0]
    assert B == 8

    if _NC is None:
        _NC = _build()
        _NC.finalize()  # run bacc passes (wait splitting, reg alloc, ACT tables)

    def prepack(w):  # [768, 128] -> [p, t, n] layout [128, 768]
        return np.ascontiguousarray(
            w.reshape(HT, P, P).transpose(1, 0, 2).reshape(P, HT * P).astype(bf16)
        )

    wqk = prepack(np.concatenate([np.asarray(Wq), np.asarray(Wk)], axis=1))
    wvv = prepack(np.concatenate([np.asarray(Wv), np.asarray(Wv)], axis=1))
    ident = np.eye(P, dtype=bf16)
    bqk = np.concatenate([np.asarray(bq), np.asarray(bk)]).astype(np.float32)
    bvv = np.concatenate([np.asarray(bv), np.asarray(bv)]).astype(np.float32)

    in_maps = []
    for b in range(B):
        mb = ((np.asarray(mask[b], np.float32) - 1.0) * 1e9).reshape(NT, P).T
        in_maps.append(
            {
                "qT": np.ascontiguousarray(np.asarray(query[b]).T.astype(bf16)),
                "kT": np.ascontiguousarray(np.asarray(key[b]).T.astype(bf16)),
                "vT": np.ascontiguousarray(np.asarray(value[b]).T.astype(bf16)),
                "wqk": wqk,
                "wvv": wvv,
                "ident": ident,
                "bqk": bqk.reshape(P, 1),
                "bvv": bvv.reshape(P, 1),
                "mb": np.ascontiguousarray(mb),
            }
        )

    res = run_bass_kernel_spmd(
        _NC,
        in_maps,
        core_ids=list(range(8)),
        trace=bool(os.environ.get("KERNEL_TRACE")),
    )
    LAST_RESULT = res
    out = np.empty((B, S, D), dtype=np.float32)
    for b in range(B):
        arr = np.asarray(res.results[b]["o"])  # [128, 1536]
        for ci in range(NCH):
            blk = arr[(ci % 2) * D : (ci % 2) * D + D,
                      (ci // 2) * CH : (ci // 2) * CH + CH]  # O^T chunk ci
            den = arr[32 * ci, 2 * CH : 3 * CH]  # denominator row
            out[b, ci * CH : (ci + 1) * CH, :] = (blk / den[None, :]).T
    return out


# revision 24
# speedup vs baseline: 1.0903x; 1.0903x over previous
"""Single-head attention (B=8, S=2048, H=768, D=64) on 8 TRN2 NeuronCores.

Data-parallel over batch: core b computes batch element b end to end; no
collectives. Host pre-transposes Q/K/V to [H, S] bf16 so every matmul
contraction lands on the partition axis.

Per-core dataflow (matmuls bf16 x bf16 -> f32 PSUM):
  warmup MMs on the identity flip the PE HAM clock-gate to 2.4 GHz while
    the first input quarters stream in.
  [qT; kT][128, 512] = [Wq|Wk]^T @ [queryT, keyT]  per 512-col quarter via
    col-packed concurrent matmul pairs; bias, then PE identity-matmuls
    duplicate qT/kT into both partition halves (qqT/kkT) so scores can
    alternate PE row groups (hides LDWEIGHTS).
  S^T tile [sk=128, sq=1024] per (t, half) in PSUM; P^T = exp(S^T/8 + mask)
    straight to bf16 SBUF (ACT engine saturated ~35us = the floor).
  O^T accumulation: per (t, half) a col-packed concurrent pair
    (M=64 each, cols 0-63 / 64-127) multiplies V^T_t against the two
    512-col chunks of P^T; denominators via 4x col-packed ones[128,32]
    matmuls accumulated over t.
  Unnormalized O^T chunks + denominator rows DMA out as [128, 1536] f32;
  host does the divide + transpose (cheap numpy).
"""

import os
from contextlib import ExitStack

import numpy as np
import ml_dtypes

import concourse.bass as bass
import concourse.mybir as mybir
import concourse.tile as tile
from concourse import bacc
from concourse.bass_utils import run_bass_kernel_spmd

S, H, D = 2048, 768, 64
P = 128
NT = S // P      # 16 sk tiles
HT = H // P      # 6 h tiles
CH = 512         # sq chunk (quarter) = matmul free dim = PSUM bank
NCH = S // CH    # 4
BF = mybir.dt.bfloat16
F32 = mybir.dt.float32
AF = mybir.ActivationFunctionType

LAST_RESULT = None  # BassKernelResults of the most recent run (for test.py)


def _build(debug=False):
    nc = bacc.Bacc()
    qT_d = nc.declare_dram_parameter("qT", [H, S], BF, isOutput=False)
    kT_d = nc.declare_dram_parameter("kT", [H, S], BF, isOutput=False)
    vT_d = nc.declare_dram_parameter("vT", [H, S], BF, isOutput=False)
    # {ident | wqk | wvv} host-prepacked into one [p, n] tensor so a single
    # contiguous DMA delivers them (each dma_start costs ~680ns of serial
    # descriptor-gen on its queue's sequencer)
    wid_d = nc.declare_dram_parameter("wid", [P, P + 2 * HT * P], BF, isOutput=False)
    # {bqk | bvv | mb} packed the same way
    cst_d = nc.declare_dram_parameter("cst", [P, 2 + NT], F32, isOutput=False)
    o_d = nc.declare_dram_parameter("o", [P, 3 * CH], F32, isOutput=True)

    with ExitStack() as ctx:
        tc = ctx.enter_context(tile.TileContext(nc))
        consts = ctx.enter_context(tc.tile_pool(name="consts", bufs=1))
        stqk = ctx.enter_context(tc.tile_pool(name="stqk", bufs=4))
        stv = ctx.enter_context(tc.tile_pool(name="stv", bufs=4))
        persist = ctx.enter_context(tc.tile_pool(name="persist", bufs=1))
        ppool = ctx.enter_context(tc.tile_pool(name="ppool", bufs=2 * NT))
        psc = ctx.enter_context(tc.tile_pool(name="psc", bufs=1, space="PSUM"))
        psw = ctx.enter_context(tc.tile_pool(name="psw", bufs=1, space="PSUM"))
        psav = ctx.enter_context(tc.tile_pool(name="psav", bufs=1, space="PSUM"))

        # ---- constants: one DMA each ----
        wid_sb = consts.tile([P, P + 2 * HT * P], BF, tag="wid")
        nc.sync.dma_start(out=wid_sb, in_=wid_d[:, :])
        ident_bf = wid_sb[:, 0:P]

        def w_half(kind, h, lo, hi):  # packed weight slice [128, hi-lo]
            base = P + kind * HT * P + h * P
            return wid_sb[:, base + lo : base + hi]

        cst_sb = consts.tile([P, 2 + NT], F32, tag="cst")
        nc.gpsimd.dma_start(out=cst_sb, in_=cst_d[:, :])
        bqk_sb = cst_sb[:, 0:1]
        bvv_sb = cst_sb[:, 1:2]
        mb_sb = cst_sb[:, 2 : 2 + NT]
        ones32 = consts.tile([P, 32], BF, tag="ones32")
        nc.vector.memset(ones32, 1.0)

        # ---- input staging: whole halves as single rearranged DMAs
        # (768 descriptors x 2KB lines each, streaming while generating).
        # Queue plan keeps every transfer off the ACT queue except q-half0
        # (whose trigger completes before the first exp could run anyway):
        #   sync:   wid, k-h0, k-h1, q-h1, v[h0-2]-h0, v[h0-2]-h1, outputs
        #   scalar: q-h0, then the exp stream
        #   gpsimd: cst, v[h3-5]-h0, v[h3-5]-h1
        st_q, st_k, st_v = {}, {}, {}
        HS = S // 2
        kst, qst, vst_a, vst_b = {}, {}, {}, {}
        for hf in range(2):
            kst[hf] = stqk.tile([P, HT, HS], BF, tag="stqk", name=f"st_k{hf}")
            nc.sync.dma_start(
                out=kst[hf],
                in_=kT_d[:, hf * HS : (hf + 1) * HS].rearrange(
                    "(t p) n -> p t n", p=P
                ),
            )
        qst[0] = stqk.tile([P, HT, HS], BF, tag="stqk", name="st_q0")
        nc.scalar.dma_start(
            out=qst[0],
            in_=qT_d[:, 0:HS].rearrange("(t p) n -> p t n", p=P),
        )
        qst[1] = stqk.tile([P, HT, HS], BF, tag="stqk", name="st_q1")
        nc.sync.dma_start(
            out=qst[1],
            in_=qT_d[:, HS : 2 * HS].rearrange("(t p) n -> p t n", p=P),
        )
        for hf in range(2):
            vst_a[hf] = stv.tile([P, 3, HS], BF, tag="stv", name=f"st_va{hf}")
            nc.sync.dma_start(
                out=vst_a[hf],
                in_=vT_d[0 : 3 * P, hf * HS : (hf + 1) * HS].rearrange(
                    "(t p) n -> p t n", p=P
                ),
            )
            vst_b[hf] = stv.tile([P, 3, HS], BF, tag="stv", name=f"st_vb{hf}")
            nc.gpsimd.dma_start(
                out=vst_b[hf],
                in_=vT_d[3 * P : 6 * P, hf * HS : (hf + 1) * HS].rearrange(
                    "(t p) n -> p t n", p=P
                ),
            )
        for c in range(NCH):
            for h in range(HT):
                st_k[h, c] = kst[c // 2][:, h, (c % 2) * CH : (c % 2 + 1) * CH]
                st_q[h, c] = qst[c // 2][:, h, (c % 2) * CH : (c % 2 + 1) * CH]
        for half in range(2):
            for h in range(HT):
                grp = vst_a if h < 3 else vst_b
                for i in range(2):
                    st_v[h, half, i] = grp[half][
                        :, h % 3, i * CH : (i + 1) * CH
                    ]

        # ---- persistent SBUF tensors ----
        qqT_sb = persist.tile([P, S], BF, tag="qqT")  # qT in both halves
        kkT_sb = persist.tile([P, S], BF, tag="kkT")  # kT in both halves
        vT2_sb = persist.tile([P, S // 2], BF, tag="vT2")  # vT chunk pairs
        vE_sb = persist.tile([P, NT * D], BF, tag="vE")  # V tiles [sk, d]
        osb = persist.tile([P, 3 * CH], F32, tag="osb")  # output staging

        # ---- PE warmup: back-to-back dummy matmuls flip the HAM clock
        # gate to 2.4 GHz during the DMA lead-in (~3.4us of activity) ----
        warm = psw.tile([P, CH], F32, tag="aux", name="warm")
        for i in range(32):
            nc.tensor.matmul(
                warm[:, :P],
                lhsT=ident_bf,
                rhs=ident_bf,
                start=True,
                stop=True,
                skip_group_check=True,
            )

        # ---- helper blocks ----
        def qk_proj_chunk(c):
            """col-packed concurrent q/k projection for sq quarter c, then
            PE identity-matmul partition duplication for that quarter."""
            pp = psav.tile(
                [P, CH], F32, tag=f"av{(c % 2) * 2}{(c % 2) * 2 + 1}",
                name=f"pp{c}",
            )
            for h in range(HT):
                nc.tensor.matmul(
                    pp[:D, :],
                    lhsT=w_half(0, h, 0, D),
                    rhs=st_q[h, c],
                    start=(h == 0),
                    stop=(h == HT - 1),
                    tile_position=(0, 0),
                    skip_group_check=True,
                )
                nc.tensor.matmul(
                    pp[D:, :],
                    lhsT=w_half(0, h, D, P),
                    rhs=st_k[h, c],
                    start=(h == 0),
                    stop=(h == HT - 1),
                    tile_position=(0, D),
                    skip_group_check=True,
                )
            ch = slice(c * CH, (c + 1) * CH)
            nc.vector.tensor_scalar_add(
                out=qqT_sb[:D, ch], in0=pp[:D, :], scalar1=bqk_sb[:D, :]
            )
            nc.vector.tensor_scalar_add(
                out=kkT_sb[D:, ch], in0=pp[D:, :], scalar1=bqk_sb[D:, :]
            )
            pd = psw.tile([P, CH], F32, tag="aux" if c % 2 else "den",
                          name=f"pd{c}")
            nc.tensor.matmul(
                pd[D:, :],
                lhsT=ident_bf[:D, :D],
                rhs=qqT_sb[:D, ch],
                start=True,
                stop=True,
                tile_position=(0, D),
                skip_group_check=True,
            )
            nc.tensor.matmul(
                pd[:D, :],
                lhsT=ident_bf[D:, D:],
                rhs=kkT_sb[D:, ch],
                start=True,
                stop=True,
                tile_position=(D, 0),
                skip_group_check=True,
            )
            nc.vector.tensor_copy(out=qqT_sb[D:, ch], in_=pd[D:, :])
            nc.vector.tensor_copy(out=kkT_sb[:D, ch], in_=pd[:D, :])

        pth = {}

        def scores_exp(t, half):
            """scores for sk-tile t over sq half (row group alternates with
            t to hide LDWEIGHTS), exp straight into a bf16 tile."""
            lo, hi = (0, D) if t % 2 == 0 else (D, P)
            ps = psc.tile([P, 2 * CH], F32, tag=f"sc{t % 2}",
                          name=f"ps{t}_{half}")
            for sub in range(2):
                c = 2 * half + sub
                nc.tensor.matmul(
                    ps[:, sub * CH : (sub + 1) * CH],
                    lhsT=kkT_sb[lo:hi, t * P : (t + 1) * P],
                    rhs=qqT_sb[lo:hi, c * CH : (c + 1) * CH],
                    start=True,
                    stop=True,
                    tile_position=(lo, 0),
                    skip_group_check=True,
                )
            pt = ppool.tile([P, 2 * CH], BF, tag="pT", name=f"pt{t}_{half}")
            nc.scalar.activation(
                out=pt,
                in_=ps,
                func=AF.Exp,
                bias=mb_sb[:, t : t + 1],
                scale=0.125,
            )
            pth[t, half] = pt

        pav = {}

        def vE_slice(t):
            """vE block layout: transpose j holds tiles (k, k+4) side by
            side; see v_proj."""
            b = (t // 8) * 4 + (t % 4)
            half = (t % 8) // 4
            return vE_sb[:, b * P + half * D : b * P + half * D + D]

        def av(t, half):
            """col-packed concurrent O^T accumulation pair for sk-tile t:
            chunk 2*half -> partitions 0:64, chunk 2*half+1 -> 64:128."""
            key = f"av{2 * half}{2 * half + 1}"
            if half not in pav:
                pav[half] = psav.tile([P, CH], F32, tag=key, name=key)
            vt = vE_slice(t)
            nc.tensor.matmul(
                pav[half][:D, :],
                lhsT=vt,
                rhs=pth[t, half][:, :CH],
                start=(t == 0),
                stop=(t == NT - 1),
                tile_position=(0, 0),
                skip_group_check=True,
            )
            nc.tensor.matmul(
                pav[half][D:, :],
                lhsT=vt,
                rhs=pth[t, half][:, CH:],
                start=(t == 0),
                stop=(t == NT - 1),
                tile_position=(0, D),
                skip_group_check=True,
            )

        pden = [None]

        def den4(t):
            """4x col-packed concurrent denominator matmuls: chunk ci's
            softmax denominator accumulates in partitions 32ci:32ci+32."""
            if pden[0] is None:
                pden[0] = psw.tile([P, CH], F32, tag="den", name="pden")
            for ci in range(NCH):
                nc.tensor.matmul(
                    pden[0][32 * ci : 32 * (ci + 1), :],
                    lhsT=ones32[:, :],
                    rhs=pth[t, ci // 2][:, (ci % 2) * CH : (ci % 2 + 1) * CH],
                    start=(t == 0),
                    stop=(t == NT - 1),
                    tile_position=(0, 32 * ci),
                    skip_group_check=True,
                )

        def v_proj(u):
            """v projection chunk-pair u (chunks 2u -> rows 0:64,
            2u+1 -> rows 64:128), then PE-transposes [128,128] blocks of
            vT2 into vE (each block = vE tiles k and k+4 side by side)."""
            pv = psw.tile([P, CH], F32, tag="den" if u == 0 else "aux",
                          name=f"pv{u}")
            for h in range(HT):
                nc.tensor.matmul(
                    pv[:D, :],
                    lhsT=w_half(1, h, 0, D),
                    rhs=st_v[h, u, 0],
                    start=(h == 0),
                    stop=(h == HT - 1),
                    tile_position=(0, 0),
                    skip_group_check=True,
                )
                nc.tensor.matmul(
                    pv[D:, :],
                    lhsT=w_half(1, h, D, P),
                    rhs=st_v[h, u, 1],
                    start=(h == 0),
                    stop=(h == HT - 1),
                    tile_position=(0, D),
                    skip_group_check=True,
                )
            nc.vector.tensor_scalar_add(
                out=vT2_sb[:, u * CH : (u + 1) * CH], in0=pv, scalar1=bvv_sb
            )
            for j in range(4):
                tag = ("aux", "den")[(j + u) % 2]
                pt = psw.tile([P, P], BF, tag=tag, name=f"ptv{u}_{j}")
                nc.tensor.transpose(
                    pt,
                    in_=vT2_sb[:, u * CH + j * P : u * CH + (j + 1) * P],
                    identity=ident_bf,
                )
                b = 4 * u + j
                nc.vector.tensor_copy(
                    out=vE_sb[:, b * P : (b + 1) * P], in_=pt
                )

        # ---- schedule (program order == Tile priority) ----
        # phase A: project q/k quarters, stream scores+exp for sq half 0,
        # slot v-projection + first-half AV under the exp umbrella.
        qk_proj_chunk(0)
        qk_proj_chunk(1)
        for t in range(4):
            scores_exp(t, 0)
        qk_proj_chunk(2)
        qk_proj_chunk(3)
        for t in range(4, 8):
            scores_exp(t, 0)
        v_proj(0)
        for t in range(4):
            av(t, 0)
        for t in range(8, 12):
            scores_exp(t, 0)
        for t in range(4, 8):
            av(t, 0)
        v_proj(1)
        for t in range(12, NT):
            scores_exp(t, 0)
        for t in range(8, NT):
            av(t, 0)

        # phase B: sq half 1 + denominators. Pairs of t keep the PE in
        # coarse bursts (scores t,t+1 back-to-back, then av+den for both)
        # so the HAM clock gate stays at 2.4 GHz.
        for tp in range(0, NT, 2):
            scores_exp(tp, 1)
            scores_exp(tp + 1, 1)
            av(tp, 1)
            den4(tp)
            av(tp + 1, 1)
            den4(tp + 1)

        # ---- epilogue: stage unnormalized O^T + denominators, DMA out;
        # the host divides and transposes ----
        nc.vector.tensor_copy(out=osb[:, 0:CH], in_=pav[0])
        nc.sync.dma_start(out=o_d[:, 0:CH], in_=osb[:, 0:CH])
        nc.vector.tensor_copy(out=osb[:, CH : 2 * CH], in_=pav[1])
        nc.vector.tensor_copy(out=osb[:, 2 * CH : 3 * CH], in_=pden[0])
        nc.sync.dma_start(out=o_d[:, CH : 3 * CH], in_=osb[:, CH : 3 * CH])

    return nc


_NC = None


def kernel(query, key, value, mask, Wq, bq, Wk, bk, Wv, bv):
    global _NC, LAST_RESULT
    bf16 = ml_dtypes.bfloat16
    B = query.shape[0]
    assert B == 8

    if _NC is None:
        _NC = _build()
        _NC.finalize()  # run bacc passes (wait splitting, reg alloc, ACT tables)

    def prepack(w):  # [768, 128] -> [p, t, n] layout [128, 768]
        return np.ascontiguousarray(
            w.reshape(HT, P, P).transpose(1, 0, 2).reshape(P, HT * P).astype(bf16)
        )

    wid = np.ascontiguousarray(
        np.concatenate(
            [
                np.eye(P, dtype=bf16),
                prepack(np.concatenate([np.asarray(Wq), np.asarray(Wk)], axis=1)),
                prepack(np.concatenate([np.asarray(Wv), np.asarray(Wv)], axis=1)),
            ],
            axis=1,
        )
    )
    bqk = np.concatenate([np.asarray(bq), np.asarray(bk)]).astype(np.float32)
    bvv = np.concatenate([np.asarray(bv), np.asarray(bv)]).astype(np.float32)

    in_maps = []
    for b in range(B):
        mb = ((np.asarray(mask[b], np.float32) - 1.0) * 1e9).reshape(NT, P).T
        cst = np.ascontiguousarray(
            np.concatenate([bqk[:, None], bvv[:, None], mb], axis=1)
        ).astype(np.float32)
        in_maps.append(
            {
                "qT": np.ascontiguousarray(np.asarray(query[b]).T.astype(bf16)),
                "kT": np.ascontiguousarray(np.asarray(key[b]).T.astype(bf16)),
                "vT": np.ascontiguousarray(np.asarray(value[b]).T.astype(bf16)),
                "wid": wid,
                "cst": cst,
            }
        )

    res = run_bass_kernel_spmd(
        _NC,
        in_maps,
        core_ids=list(range(8)),
        trace=bool(os.environ.get("KERNEL_TRACE")),
    )
    LAST_RESULT = res
    out = np.empty((B, S, D), dtype=np.float32)
    for b in range(B):
        arr = np.asarray(res.results[b]["o"])  # [128, 1536]
        for ci in range(NCH):
            blk = arr[(ci % 2) * D : (ci % 2) * D + D,
                      (ci // 2) * CH : (ci // 2) * CH + CH]  # O^T chunk ci
            den = arr[32 * ci, 2 * CH : 3 * CH]  # denominator row
            out[b, ci * CH : (ci + 1) * CH, :] = (blk / den[None, :]).T
    return out
